# revision 1
# baseline (speedup 1.0000x reference)
"""Trainium2 Bass kernel for nn_NNModel2 (2x NNConv GNN + pooled MLP readout).

Self-contained: accepts FULL inputs, shards edges across 8 NeuronCores
(edge-parallel, node-aligned ownership by dst), runs one SPMD Bass program,
returns the FULL [256, 1] output.

Math (per NNConv layer, aggr='add'):
    w_e  = (edge_attr @ nn_w + nn_b).reshape(E, I, O)
    msg  = einsum('ei,eio->eo', x[src], w_e)
    out  = segment_sum(msg, dst, N) + x @ root_w + bias
restructured as one dense matmul over z:
    z[e, (k,i)] = edge_attr[e,k] * x[src[e], i]
    msg = z @ W' + x[src] @ B';  W'[(k,i), o] = nn_w[k, i*O+o]
Scatter-add and graph pooling are one-hot matmuls (is_equal vs iota consts).
conv1 -> AllGather h1 (bf16) -> conv2 -> pooled partials -> AllReduce -> MLP.
"""

import sys

sys.path.insert(0, "/opt/trn_rl_repo")

import numpy as np

from concourse import bacc, bass, mybir
import concourse.tile as tile
from concourse import bass_utils

P = 128
NCORES = 8
N_NODES = 4096
N_EDGES = 8192
N_GRAPHS = 256
DN = 64
DE = 32
H = 256
NSH = N_NODES // NCORES  # 512
NT = NSH // P  # 4
GT = N_GRAPHS // P  # 2

F32 = mybir.dt.float32
BF16 = mybir.dt.bfloat16
I16 = mybir.dt.int16
AF = mybir.ActivationFunctionType
ALU = mybir.AluOpType

_cache = {}


def _wrap_idx(idx, n):
    idx = np.asarray(idx, dtype=np.int16)
    assert idx.shape == (n,) and n % 16 == 0
    return np.tile(idx.reshape(n // 16, 16).T, (8, 1)).copy()


def _build(e_pad, upto="full"):
    ET = e_pad // P
    nc = bacc.Bacc(num_devices=NCORES)

    x = nc.dram_tensor("x", [N_NODES, DN], F32, kind="ExternalInput")
    attr = nc.dram_tensor("attr", [N_EDGES, DE], F32, kind="ExternalInput")
    nn1_w = nc.dram_tensor("nn1_w", [DE, DN * H], F32, kind="ExternalInput")
    nn1_b = nc.dram_tensor("nn1_b", [1, DN * H], F32, kind="ExternalInput")
    r1w = nc.dram_tensor("r1w", [DN, H], F32, kind="ExternalInput")
    b1 = nc.dram_tensor("b1", [1, H], F32, kind="ExternalInput")
    nn2_w = nc.dram_tensor("nn2_w", [DE, H * H], F32, kind="ExternalInput")
    nn2_b = nc.dram_tensor("nn2_b", [1, H * H], F32, kind="ExternalInput")
    r2w = nc.dram_tensor("r2w", [H, H], F32, kind="ExternalInput")
    b2 = nc.dram_tensor("b2", [1, H], F32, kind="ExternalInput")
    l1w = nc.dram_tensor("l1w", [H, H // 2], F32, kind="ExternalInput")
    l1b = nc.dram_tensor("l1b", [H // 2, 1], F32, kind="ExternalInput")
    l2w = nc.dram_tensor("l2w", [H // 2, 1], F32, kind="ExternalInput")
    l2b = nc.dram_tensor("l2b", [1, 1], F32, kind="ExternalInput")
    src_w = nc.dram_tensor("src_w", [P, e_pad // 16], I16, kind="ExternalInput")
    eid_w = nc.dram_tensor("eid_w", [P, e_pad // 16], I16, kind="ExternalInput")
    node_w = nc.dram_tensor("node_w", [P, NSH // 16], I16, kind="ExternalInput")
    dstl = nc.dram_tensor("dstl", [e_pad, 1], F32, kind="ExternalInput")
    batchl = nc.dram_tensor("batchl", [NSH, 1], F32, kind="ExternalInput")
    iota512 = nc.dram_tensor("iota512", [P, NSH], F32, kind="ExternalInput")
    iotag = nc.dram_tensor("iotag", [P, N_GRAPHS], F32, kind="ExternalInput")
    ident = nc.dram_tensor("ident", [P, P], F32, kind="ExternalInput")
    out = nc.dram_tensor("out", [N_GRAPHS, 1], F32, kind="ExternalOutput")

    def dbg_out(name, shape):
        return nc.dram_tensor(name, shape, F32, kind="ExternalOutput")

    rg = [list(range(NCORES))]
    ST = {"w": 1, "gather": 1, "msg1": 2, "h1": 2, "ag": 3, "h2": 4, "full": 99}[upto]

    with tile.TileContext(nc, num_cores=NCORES) as tc:
        with (
            tc.tile_pool(name="const", bufs=1) as cp,
            tc.tile_pool(name="work", bufs=3) as wp,
            tc.tile_pool(name="dram", bufs=1, space="DRAM") as dr,
        ):
            # ======== stage 0: resident weights (bf16) + bf16 DRAM tables
            w2sb = cp.tile([P, 2 * DE, H], BF16)
            w2_src = nn2_w.rearrange("k (h p o) -> p (k h) o", h=2, p=P, o=H)
            with tc.tile_pool(name="staging", bufs=2) as stp:
                w1sb = cp.tile([P, 16, H], BF16)
                w1_src = nn1_w.rearrange("(t k2) (i o) -> (k2 i) t o", k2=2, o=H)
                for c in range(2):
                    st1 = stp.tile([P, 8, H], F32, tag="w2st", name=f"w1st{c}")
                    nc.sync.dma_start(out=st1[:], in_=w1_src[:, 8 * c : 8 * (c + 1), :])
                    nc.scalar.activation(
                        out=w1sb[:, 8 * c : 8 * (c + 1), :], in_=st1[:], func=AF.Copy
                    )

                def load_bf(dst_tile, src_ap, tag="bst"):
                    sst = stp.tile(
                        list(src_ap.shape), F32, tag=tag,
                        name=f"st_{dst_tile.tensor.name}",
                    )
                    nc.sync.dma_start(out=sst[:], in_=src_ap)
                    nc.vector.tensor_copy(out=dst_tile[:], in_=sst[:])

                b1p = cp.tile([DN, H], BF16)
                load_bf(b1p, nn1_b.rearrange("one (i o) -> (one i) o", o=H))
                b2p = cp.tile([P, 2, H], BF16)
                load_bf(b2p, nn2_b.rearrange("one (h p o) -> (one p) h o", h=2, p=P, o=H))
                r1wb = cp.tile([DN, H], BF16)
                load_bf(r1wb, r1w[:])
                r2wb = cp.tile([P, 2, H], BF16)
                load_bf(r2wb, r2w.rearrange("(h p) o -> p h o", p=P))
                l1wb = cp.tile([P, 2, H // 2], BF16)
                load_bf(l1wb, l1w.rearrange("(h p) m -> p h m", p=P))
                l2wb = cp.tile([H // 2, 1], BF16)
                load_bf(l2wb, l2w[:], tag="bst2")
                identb = cp.tile([P, P], BF16)
                load_bf(identb, ident[:])

                b1sb = cp.tile([1, H], F32)
                nc.sync.dma_start(out=b1sb[:], in_=b1[:])
                b2sb = cp.tile([1, H], F32)
                nc.sync.dma_start(out=b2sb[:], in_=b2[:])
                l1bsb = cp.tile([H // 2, 1], F32)
                nc.sync.dma_start(out=l1bsb[:], in_=l1b[:])
                l2bsb = cp.tile([1, 1], F32)
                nc.sync.dma_start(out=l2bsb[:], in_=l2b[:])
                ones = cp.tile([1, P], F32)
                nc.vector.memset(ones[:], 1.0)
                io512 = cp.tile([P, NSH], F32)
                nc.sync.dma_start(out=io512[:], in_=iota512[:])
                iog = cp.tile([P, N_GRAPHS], F32)
                nc.sync.dma_start(out=iog[:], in_=iotag[:])
                dstl_sb = cp.tile([P, ET, 1], F32)
                nc.sync.dma_start(
                    out=dstl_sb[:], in_=dstl.rearrange("(e p) one -> p e one", p=P)
                )
                batchl_sb = cp.tile([P, NT, 1], F32)
                nc.sync.dma_start(
                    out=batchl_sb[:], in_=batchl.rearrange("(t p) one -> p t one", p=P)
                )
                srcw_sb = cp.tile([P, e_pad // 16], I16)
                nc.sync.dma_start(out=srcw_sb[:], in_=src_w[:])
                eidw_sb = cp.tile([P, e_pad // 16], I16)
                nc.sync.dma_start(out=eidw_sb[:], in_=eid_w[:])
                nodew_sb = cp.tile([P, NSH // 16], I16)
                nc.sync.dma_start(out=nodew_sb[:], in_=node_w[:])

                x_dup = dr.tile([N_NODES, P], BF16)
                stx = stp.tile([P, N_NODES // P, DN], F32, tag="xst", bufs=1)
                nc.sync.dma_start(out=stx[:], in_=x.rearrange("(nb p) d -> p nb d", p=P))
                xbf = stp.tile([P, N_NODES // P, DN], BF16, tag="xbf", bufs=1)
                nc.vector.tensor_copy(out=xbf[:], in_=stx[:])
                x_dup_v = x_dup[:].rearrange("(nb p) c -> p nb c", p=P)
                nc.sync.dma_start(out=x_dup_v[:, :, 0:DN], in_=xbf[:])
                nc.sync.dma_start(out=x_dup_v[:, :, DN : 2 * DN], in_=xbf[:])

                attr_pad = dr.tile([N_EDGES, P], BF16)
                sta = stp.tile([P, N_EDGES // P, DE], F32, tag="xst", bufs=1)
                nc.sync.dma_start(
                    out=sta[:], in_=attr.rearrange("(nb p) d -> p nb d", p=P)
                )
                apd = stp.tile([P, N_EDGES // P, DE], BF16, tag="apd", bufs=1)
                nc.vector.tensor_copy(out=apd[:], in_=sta[:])
                nc.sync.dma_start(
                    out=attr_pad[:].rearrange("(nb p) c -> p nb c", p=P)[:, :, 0:DE],
                    in_=apd[:],
                )

                # W2 last: only needed at conv2; let gather-chain DMAs go first
                for c in range(8):
                    st = stp.tile([P, 8, H], F32, tag="w2st", name=f"w2st{c}")
                    nc.sync.dma_start(out=st[:], in_=w2_src[:, 8 * c : 8 * (c + 1), :])
                    nc.scalar.activation(
                        out=w2sb[:, 8 * c : 8 * (c + 1), :], in_=st[:], func=AF.Copy
                    )

            # ======== stage 1: gathers + attr broadcast tiles
            with tc.tile_pool(name="big", bufs=1) as bp:
                attrT = cp.tile([P, 1, e_pad], BF16)
                nc.gpsimd.dma_gather(
                    out_ap=attrT[:], in_ap=attr_pad[:], idxs_ap=eidw_sb[:],
                    num_idxs=e_pad, num_idxs_reg=e_pad, elem_size=P, transpose=True, single_packet=False,
                )
                attrT_dram = dr.tile([DE, e_pad], BF16)
                nc.sync.dma_start(out=attrT_dram[:], in_=attrT[0:DE, 0, :])

                xsrcT = cp.tile([P, 1, e_pad], BF16)
                nc.gpsimd.dma_gather(
                    out_ap=xsrcT[:], in_ap=x_dup[:], idxs_ap=srcw_sb[:],
                    num_idxs=e_pad, num_idxs_reg=e_pad, elem_size=P, transpose=True, single_packet=False,
                )
                xshT = cp.tile([P, 1, NSH], BF16)
                nc.gpsimd.dma_gather(
                    out_ap=xshT[:], in_ap=x_dup[:], idxs_ap=nodew_sb[:],
                    num_idxs=NSH, num_idxs_reg=NSH, elem_size=P, transpose=True, single_packet=False,
                )

                bc_all = bp.tile([P, DE, e_pad], BF16, name="bc_all")
                for kc in range(4):
                    nc.sync.dma_start(
                        out=bc_all[:, 8 * kc : 8 * (kc + 1), :],
                        in_=attrT_dram[8 * kc : 8 * (kc + 1), :].partition_broadcast(P),
                    )

                if upto == "w":
                    dw1 = dbg_out("d_w1", [P, 16 * H])
                    for j in range(2):
                        tw = wp.tile([P, 8, H], F32, tag="dbgw")
                        nc.vector.tensor_copy(out=tw[:], in_=w1sb[:, 8*j:8*(j+1), :])
                        nc.sync.dma_start(
                            out=dw1[:].rearrange("p (t o) -> p t o", o=H)[:, 8*j:8*(j+1), :],
                            in_=tw[:])
                    dw2 = dbg_out("d_w2", [P, 4 * H])
                    tw2 = wp.tile([P, 4, H], F32, tag="dbgw2")
                    nc.vector.tensor_copy(out=tw2[:], in_=w2sb[:, 0:4, :])
                    nc.sync.dma_start(
                        out=dw2[:].rearrange("p (t o) -> p t o", o=H), in_=tw2[:])

                if ST == 1 and upto == "gather":
                    d1 = dbg_out("d_xsrcT", [P, e_pad])
                    tmp = wp.tile([P, e_pad], F32, tag="dbgf")
                    nc.vector.tensor_copy(out=tmp[:], in_=xsrcT[:, 0, :])
                    nc.sync.dma_start(out=d1[:], in_=tmp[:])
                    d2 = dbg_out("d_attrT", [DE, e_pad])
                    tmp2 = wp.tile([DE, e_pad], F32, tag="dbg2")
                    nc.vector.tensor_copy(out=tmp2[:], in_=attrT[0:DE, 0, :])
                    nc.sync.dma_start(out=d2[:], in_=tmp2[:])
                    d3 = dbg_out("d_bc5", [P, e_pad])
                    tmp3 = wp.tile([P, e_pad], F32, tag="dbgf")
                    nc.vector.tensor_copy(out=tmp3[:], in_=bc_all[:, 5, :])
                    nc.sync.dma_start(out=d3[:], in_=tmp3[:])

                if ST >= 2:
                    with tc.tile_pool(name="psA", bufs=1, space="PSUM") as psA:
                        # ======== stage 2: conv1
                        msg_ps = [
                            psA.tile([P, 2 * H], F32, space="PSUM",
                                     tag=f"msg{j}", name=f"msg1_{j}")
                            for j in range((ET + 1) // 2)
                        ]

                        def m1(e):
                            return msg_ps[e // 2][:, (e % 2) * H : (e % 2) * H + H]

                        for t in range(16):
                            k0, k1 = 2 * t, 2 * t + 1
                            zt = wp.tile([P, e_pad], BF16, tag="zt", bufs=4)
                            nc.vector.tensor_tensor(
                                out=zt[0:DN, :], in0=xsrcT[0:DN, 0, :],
                                in1=bc_all[0:DN, k0, :], op=ALU.mult,
                            )
                            nc.vector.tensor_tensor(
                                out=zt[DN:P, :], in0=xsrcT[DN:P, 0, :],
                                in1=bc_all[DN:P, k1, :], op=ALU.mult,
                            )
                            for e in range(ET):
                                nc.tensor.matmul(
                                    m1(e), lhsT=zt[:, P * e : P * (e + 1)],
                                    rhs=w1sb[:, t, :],
                                    start=(t == 0 and e % 2 == 0), stop=False,
                                    skip_group_check=True,
                                )
                        for e in range(ET):
                            nc.tensor.matmul(
                                m1(e), lhsT=xsrcT[0:DN, 0, P * e : P * (e + 1)],
                                rhs=b1p[:], start=False, stop=True,
                                skip_group_check=True,
                            )

                        if upto == "msg1":
                            dz = dbg_out("d_z0", [P, e_pad])
                            zt0 = wp.tile([P, e_pad], BF16, tag="zt")
                            nc.vector.tensor_tensor(
                                out=zt0[0:DN, :], in0=xsrcT[0:DN, 0, :],
                                in1=bc_all[0:DN, 0, :], op=ALU.mult)
                            nc.vector.tensor_tensor(
                                out=zt0[DN:P, :], in0=xsrcT[DN:P, 0, :],
                                in1=bc_all[DN:P, 1, :], op=ALU.mult)
                            tmpz = wp.tile([P, e_pad], F32, tag="dbgf")
                            nc.vector.tensor_copy(out=tmpz[:], in_=zt0[:])
                            nc.sync.dma_start(out=dz[:], in_=tmpz[:])
                            dm = dbg_out("d_msg1", [P, ET * H])
                            for j in range((ET + 1) // 2):
                                w = min(2 * H, (ET - 2 * j) * H)
                                tmpm = wp.tile([P, 2 * H], F32, tag="dbgm")
                                nc.scalar.activation(
                                    out=tmpm[:, 0:w], in_=msg_ps[j][:, 0:w],
                                    func=AF.Copy)
                                nc.sync.dma_start(
                                    out=dm[:, 2 * H * j : 2 * H * j + w],
                                    in_=tmpm[:, 0:w])

                        agg_ps = [
                            psA.tile([P, 2 * H], F32, space="PSUM",
                                     tag=f"agg{j}", name=f"agg1_{j}")
                            for j in range(NT // 2)
                        ]

                        def a1(n):
                            return agg_ps[n // 2][:, (n % 2) * H : (n % 2) * H + H]

                        msbs = []
                        for j in range((ET + 1) // 2) if upto != "msg1" else []:
                            w = min(2 * H, (ET - 2 * j) * H)
                            msb = wp.tile([P, 2 * H], BF16, tag="msb")
                            nc.scalar.activation(
                                out=msb[:, 0:w], in_=msg_ps[j][:, 0:w], func=AF.Copy
                            )
                            msbs.append(msb)
                        for e in range(ET) if upto != "msg1" else []:
                            for n in range(NT):
                                oh = wp.tile([P, P], BF16, tag="oh", bufs=6)
                                nc.vector.tensor_scalar(
                                    out=oh[:], in0=io512[:, P * n : P * (n + 1)],
                                    scalar1=dstl_sb[:, e, :1], scalar2=None,
                                    op0=ALU.is_equal,
                                )
                                nc.tensor.matmul(
                                    a1(n), lhsT=oh[:],
                                    rhs=msbs[e // 2][:, (e % 2) * H : (e % 2) * H + H],
                                    start=(e == 0 and n % 2 == 0), stop=False,
                                    skip_group_check=True,
                                )
                        for n in range(NT) if upto != "msg1" else []:
                            nc.tensor.matmul(
                                a1(n), lhsT=xshT[0:DN, 0, P * n : P * (n + 1)],
                                rhs=r1wb[:], start=False, stop=False,
                                skip_group_check=True,
                            )
                            nc.tensor.matmul(
                                a1(n), lhsT=ones[:], rhs=b1sb[:],
                                start=False, stop=True, skip_group_check=True,
                            )
                        h1sb = bp.tile([P, NT, H], BF16)
                        for j in range(NT // 2) if upto != "msg1" else []:
                            nc.scalar.activation(
                                out=h1sb[:, 2 * j : 2 * j + 2, :],
                                in_=agg_ps[j][:, 0 : 2 * H], func=AF.Relu,
                            )

                        if ST == 2 and upto == "h1":
                            dh = dbg_out("d_h1", [P, NT * H])
                            tmp = wp.tile([P, NT, H], F32, tag="dbgf")
                            nc.vector.tensor_copy(out=tmp[:], in_=h1sb[:])
                            nc.sync.dma_start(
                                out=dh[:].rearrange("p (t o) -> p t o", o=H),
                                in_=tmp[:],
                            )

                        if ST >= 3:
                            h1cc = dr.tile([NSH, H], BF16)
                            nc.sync.dma_start(
                                out=h1cc[:].rearrange("(t p) o -> p t o", p=P),
                                in_=h1sb[:],
                            )
                            h1_all = dr.tile([N_NODES, H], BF16, addr_space="Shared")
                            nc.gpsimd.collective_compute(
                                "AllGather", ALU.bypass, replica_groups=rg,
                                ins=[h1cc[:].opt()], outs=[h1_all[:].opt()],
                            )
                        if ST == 3:
                            dh = dbg_out("d_h1all", [P, (N_NODES // P) * H])
                            stg = bp.tile([P, N_NODES // P, H], BF16)
                            nc.sync.dma_start(
                                out=stg[:],
                                in_=h1_all[:].rearrange("(nb p) o -> p nb o", p=P),
                            )
                            for nb in range(N_NODES // P):
                                tmpg = wp.tile([P, H], F32, tag="dbgf")
                                nc.vector.tensor_copy(out=tmpg[:], in_=stg[:, nb, :])
                                nc.sync.dma_start(
                                    out=dh[:, H * nb : H * (nb + 1)], in_=tmpg[:]
                                )

                        if ST >= 4:
                            # ======== stage 3+4: conv2
                            h1srcT = bp.tile([P, 2, e_pad], BF16)
                            nc.gpsimd.dma_gather(
                                out_ap=h1srcT[:], in_ap=h1_all[:], idxs_ap=srcw_sb[:],
                                num_idxs=e_pad, num_idxs_reg=e_pad, elem_size=H,
                                transpose=True, single_packet=False,
                            )
                            h1shT = bp.tile([P, 2, NSH], BF16)
                            nc.gpsimd.dma_gather(
                                out_ap=h1shT[:], in_ap=h1_all[:], idxs_ap=nodew_sb[:],
                                num_idxs=NSH, num_idxs_reg=NSH, elem_size=H,
                                transpose=True, single_packet=False,
                            )

                            msg2_ps = [
                                psA.tile([P, 2 * H], F32, space="PSUM",
                                         tag=f"msg{j}", name=f"msg2_{j}")
                                for j in range((ET + 1) // 2)
                            ]

                            def m2(e):
                                return msg2_ps[e // 2][:, (e % 2) * H : (e % 2) * H + H]

                            for t in range(64):
                                k, ih = t // 2, t % 2
                                zt = wp.tile([P, e_pad], BF16, tag="zt", bufs=4)
                                nc.vector.tensor_tensor(
                                    out=zt[:], in0=h1srcT[:, ih, :], in1=bc_all[:, k, :],
                                    op=ALU.mult,
                                )
                                for e in range(ET):
                                    nc.tensor.matmul(
                                        m2(e), lhsT=zt[:, P * e : P * (e + 1)],
                                        rhs=w2sb[:, t, :],
                                        start=(t == 0 and e % 2 == 0), stop=False,
                                        skip_group_check=True,
                                    )
                            for e in range(ET):
                                for ih in range(2):
                                    nc.tensor.matmul(
                                        m2(e),
                                        lhsT=h1srcT[:, ih, P * e : P * (e + 1)],
                                        rhs=b2p[:, ih, :], start=False,
                                        stop=(ih == 1), skip_group_check=True,
                                    )

                            agg2_ps = [
                                psA.tile([P, 2 * H], F32, space="PSUM",
                                         tag=f"agg{j}", name=f"agg2_{j}")
                                for j in range(NT // 2)
                            ]

                            def a2(n):
                                return agg2_ps[n // 2][:, (n % 2) * H : (n % 2) * H + H]

                            msbs2 = []
                            for j in range((ET + 1) // 2):
                                w = min(2 * H, (ET - 2 * j) * H)
                                msb = wp.tile([P, 2 * H], BF16, tag="msb")
                                nc.scalar.activation(
                                    out=msb[:, 0:w], in_=msg2_ps[j][:, 0:w],
                                    func=AF.Copy,
                                )
                                msbs2.append(msb)
                            for e in range(ET):
                                for n in range(NT):
                                    oh = wp.tile([P, P], BF16, tag="oh", bufs=6)
                                    nc.vector.tensor_scalar(
                                        out=oh[:], in0=io512[:, P * n : P * (n + 1)],
                                        scalar1=dstl_sb[:, e, :1], scalar2=None,
                                        op0=ALU.is_equal,
                                    )
                                    nc.tensor.matmul(
                                        a2(n), lhsT=oh[:],
                                        rhs=msbs2[e // 2][:, (e % 2) * H : (e % 2) * H + H],
                                        start=(e == 0 and n % 2 == 0), stop=False,
                                        skip_group_check=True,
                                    )
                            for n in range(NT):
                                for kh in range(2):
                                    nc.tensor.matmul(
                                        a2(n),
                                        lhsT=h1shT[:, kh, P * n : P * (n + 1)],
                                        rhs=r2wb[:, kh, :], start=False, stop=False,
                                        skip_group_check=True,
                                    )
                                nc.tensor.matmul(
                                    a2(n), lhsT=ones[:], rhs=b2sb[:],
                                    start=False, stop=True, skip_group_check=True,
                                )
                            h2e = bp.tile([P, NT, H + 1], BF16)
                            nc.vector.memset(h2e[:, :, H : H + 1], 1.0)
                            for j in range(NT // 2):
                                nc.scalar.activation(
                                    out=h2e[:, 2 * j : 2 * j + 2, 0:H],
                                    in_=agg2_ps[j][:, 0 : 2 * H], func=AF.Copy,
                                )

                        if ST == 4:
                            dh = dbg_out("d_h2", [P, NT * H])
                            tmp = wp.tile([P, NT, H], F32, tag="dbgf")
                            for n in range(NT):
                                nc.vector.tensor_copy(
                                    out=tmp[:, n, :], in_=h2e[:, n, 0:H]
                                )
                            nc.sync.dma_start(
                                out=dh[:].rearrange("p (t o) -> p t o", o=H),
                                in_=tmp[:],
                            )

                        if ST >= 5:
                            # ======== stage 5: pooling
                            pool_ps = [
                                psA.tile([P, 2 * H], F32, space="PSUM",
                                         tag=f"agg{g}", name=f"pool_{g}")
                                for g in range(GT)
                            ]
                            for n in range(NT):
                                for g in range(GT):
                                    ohg = wp.tile([P, P], BF16, tag="oh", bufs=6)
                                    nc.vector.tensor_scalar(
                                        out=ohg[:], in0=iog[:, P * g : P * (g + 1)],
                                        scalar1=batchl_sb[:, n, :1], scalar2=None,
                                        op0=ALU.is_equal,
                                    )
                                    nc.tensor.matmul(
                                        pool_ps[g][:, 0 : H + 1], lhsT=ohg[:],
                                        rhs=h2e[:, n, :], start=(n == 0),
                                        stop=(n == NT - 1),
                                        skip_group_check=(n not in (0, NT - 1)),
                                    )
                            plsb = bp.tile([P, GT, H + 1], F32)
                            for g in range(GT):
                                nc.scalar.activation(
                                    out=plsb[:, g, :], in_=pool_ps[g][:, 0 : H + 1],
                                    func=AF.Copy,
                                )
                            pcc_in = dr.tile([N_GRAPHS, H + 1], F32)
                            nc.sync.dma_start(
                                out=pcc_in[:].rearrange("(g p) c -> p g c", p=P),
                                in_=plsb[:],
                            )
                            pcc_out = dr.tile([N_GRAPHS, H + 1], F32, addr_space="Shared")
                            nc.gpsimd.collective_compute(
                                "AllReduce", ALU.add, replica_groups=rg,
                                ins=[pcc_in[:].opt()], outs=[pcc_out[:].opt()],
                            )

                if ST >= 5:
                    # ======== stage 6: readout MLP (every core, redundant)
                    with tc.tile_pool(name="psB", bufs=1, space="PSUM") as psB:
                        pl = bp.tile([P, GT, H + 1], F32)
                        nc.sync.dma_start(
                            out=pl[:],
                            in_=pcc_out[:].rearrange("(g p) c -> p g c", p=P),
                        )
                        rec = bp.tile([P, GT, 1], F32)
                        cnt = wp.tile([P, GT, 1], F32, tag="cnt")
                        nc.vector.tensor_scalar_max(cnt[:], pl[:, :, H : H + 1], 1.0)
                        for g in range(GT):
                            nc.vector.reciprocal(out=rec[:, g, :], in_=cnt[:, g, :])
                        mean_bf = bp.tile([P, GT, H], BF16)
                        for g in range(GT):
                            nc.vector.tensor_scalar(
                                out=mean_bf[:, g, :], in0=pl[:, g, 0:H],
                                scalar1=rec[:, g, :1], scalar2=None, op0=ALU.mult,
                            )
                        poolT = bp.tile([P, 2, N_GRAPHS], BF16)
                        for g in range(GT):
                            for hh in range(2):
                                tp = psB.tile([P, P], BF16, space="PSUM", tag="tp")
                                nc.tensor.transpose(
                                    out=tp[:],
                                    in_=mean_bf[:, g, P * hh : P * (hh + 1)],
                                    identity=identb[:],
                                )
                                nc.scalar.activation(
                                    out=poolT[:, hh, P * g : P * (g + 1)],
                                    in_=tp[:], func=AF.Copy,
                                )
                        z1_ps = psB.tile([P, N_GRAPHS], F32, space="PSUM", tag="z1")
                        for kh in range(2):
                            nc.tensor.matmul(
                                z1_ps[:], lhsT=l1wb[:, kh, :], rhs=poolT[:, kh, :],
                                start=(kh == 0), stop=(kh == 1),
                            )
                        z1sb = bp.tile([P, N_GRAPHS], BF16)
                        nc.scalar.activation(
                            out=z1sb[:], in_=z1_ps[:], func=AF.Relu, bias=l1bsb[:, :1]
                        )
                        o_ps = psB.tile([1, N_GRAPHS], F32, space="PSUM", tag="op")
                        nc.tensor.matmul(
                            o_ps[:], lhsT=l2wb[:], rhs=z1sb[:], start=True, stop=True
                        )
                        osb = bp.tile([1, N_GRAPHS], F32)
                        nc.scalar.activation(
                            out=osb[:], in_=o_ps[:], func=AF.Sigmoid, bias=l2bsb[:, :1]
                        )
                        nc.sync.dma_start(
                            out=out[:].rearrange("g one -> one g"), in_=osb[:]
                        )

    nc.compile()
    return nc


def _prep_inputs(inputs, e_pad=None):
    x = np.asarray(inputs["x"], dtype=np.float32)
    ei = np.asarray(inputs["edge_index"])
    attr = np.asarray(inputs["edge_attr"], dtype=np.float32)
    batch = np.asarray(inputs["batch"])
    src, dst = ei[0].astype(np.int64), ei[1].astype(np.int64)

    owner = dst // NSH
    per_core = [np.nonzero(owner == c)[0] for c in range(NCORES)]
    need = max(max(len(e) for e in per_core), 1)
    if e_pad is None:
        e_pad = max(((need + P - 1) // P) * P, P)
    assert need <= e_pad

    common = {
        "x": x,
        "attr": attr,
        "nn1_w": np.asarray(inputs["nn1_w"], dtype=np.float32),
        "nn1_b": np.asarray(inputs["nn1_b"], dtype=np.float32).reshape(1, -1),
        "r1w": np.asarray(inputs["root1_w"], dtype=np.float32),
        "b1": np.asarray(inputs["bias1"], dtype=np.float32).reshape(1, -1),
        "nn2_w": np.asarray(inputs["nn2_w"], dtype=np.float32),
        "nn2_b": np.asarray(inputs["nn2_b"], dtype=np.float32).reshape(1, -1),
        "r2w": np.asarray(inputs["root2_w"], dtype=np.float32),
        "b2": np.asarray(inputs["bias2"], dtype=np.float32).reshape(1, -1),
        "l1w": np.asarray(inputs["lin1_w"], dtype=np.float32),
        "l1b": np.asarray(inputs["lin1_b"], dtype=np.float32).reshape(-1, 1),
        "l2w": np.asarray(inputs["lin2_w"], dtype=np.float32),
        "l2b": np.asarray(inputs["lin2_b"], dtype=np.float32).reshape(1, 1),
        "iota512": np.tile(np.arange(NSH, dtype=np.float32), (P, 1)),
        "iotag": np.tile(np.arange(N_GRAPHS, dtype=np.float32), (P, 1)),
        "ident": np.eye(P, dtype=np.float32),
    }

    in_maps = []
    for c in range(NCORES):
        eids = per_core[c]
        ne = len(eids)
        src_c = np.zeros(e_pad, dtype=np.int16)
        src_c[:ne] = src[eids]
        eid_c = np.zeros(e_pad, dtype=np.int16)
        eid_c[:ne] = eids
        dstl_c = np.full(e_pad, -1.0, dtype=np.float32)
        dstl_c[:ne] = (dst[eids] - c * NSH).astype(np.float32)
        node_c = np.arange(c * NSH, (c + 1) * NSH, dtype=np.int16)
        batch_c = batch[c * NSH : (c + 1) * NSH].astype(np.float32)
        m = dict(common)
        m["src_w"] = _wrap_idx(src_c, e_pad)
        m["eid_w"] = _wrap_idx(eid_c, e_pad)
        m["node_w"] = _wrap_idx(node_c, NSH)
        m["dstl"] = dstl_c.reshape(-1, 1)
        m["batchl"] = batch_c.reshape(-1, 1)
        in_maps.append(m)
    return e_pad, in_maps


def kernel(**inputs) -> np.ndarray:
    e_pad, in_maps = _prep_inputs(inputs)
    if e_pad not in _cache:
        _cache[e_pad] = _build(e_pad)
    nc = _cache[e_pad]
    res = bass_utils.run_bass_kernel_spmd(nc, in_maps, core_ids=list(range(NCORES)))
    return np.asarray(res.results[0]["out"], dtype=np.float32)


def run_debug(upto, **inputs):
    e_pad, in_maps = _prep_inputs(inputs)
    nc = _build(e_pad, upto=upto)
    res = bass_utils.run_bass_kernel_spmd(nc, in_maps, core_ids=list(range(NCORES)))
    return e_pad, res



# revision 12
# speedup vs baseline: 1.8756x; 1.8756x over previous
"""Trainium2 Bass kernel for nn_NNModel2 (2x NNConv GNN + pooled MLP readout).

Self-contained: accepts FULL inputs, returns the FULL [256, 1] output.

Dual edge sharding over a balanced node permutation (512 nodes/core):
  - conv1: edges sharded by dst-owner. z1 = attr (x) x[src] is host-prepared
    in lhsT layout, so conv1 is pure matmuls; local one-hot scatter gives
    exact h1 (+relu) per core with NO collective.
  - conv2: edges sharded by src-owner, so h1 is already local. Messages
    scatter into a PARTIAL h2 over all 4096 (internal) nodes via dst-sorted
    windowed one-hot matmuls; partial graph pooling; one ReduceScatter of
    the [256, 257] pooled partials gives each core 32 exact graph rows; the
    readout MLP runs per-core and the host concatenates the 8 [32,1] shards.

All heavy tables stream to SBUF as bf16 just-in-time in DMA program order.
"""

import sys

sys.path.insert(0, "/opt/trn_rl_repo")

import numpy as np
import ml_dtypes

from concourse import bacc, mybir
import concourse.tile as tile
from concourse import bass_utils

P = 128
NCORES = 8
N_NODES = 4096
N_EDGES = 8192
N_GRAPHS = 256
DN = 64
DE = 32
H = 256
NSH = N_NODES // NCORES  # 512
W2 = 8  # conv2 scatter window width in n-tiles
NT = N_NODES // P  # 32 n-tiles (internal numbering)

F32 = mybir.dt.float32
BF16 = mybir.dt.bfloat16
F16 = mybir.dt.float16
AF = mybir.ActivationFunctionType
ALU = mybir.AluOpType
BF = ml_dtypes.bfloat16

_cache = {}


# ------------------------------------------------------------------ host prep
def _balance_nodes(src, dst):
    indeg = np.bincount(dst, minlength=N_NODES)
    outdeg = np.bincount(src, minlength=N_NODES)
    order = np.argsort(-(indeg + outdeg), kind="stable")
    counts = np.zeros(NCORES, dtype=np.int64)
    loads_in = np.zeros(NCORES, dtype=np.int64)
    loads_out = np.zeros(NCORES, dtype=np.int64)
    assign = np.empty(N_NODES, dtype=np.int64)
    for n in order:
        best, bestscore = -1, None
        for c in range(NCORES):
            if counts[c] >= NSH:
                continue
            score = (
                max(loads_in[c] + indeg[n], loads_out[c] + outdeg[n]),
                loads_in[c] + indeg[n] + loads_out[c] + outdeg[n],
            )
            if bestscore is None or score < bestscore:
                best, bestscore = c, score
        assign[n] = best
        counts[best] += 1
        loads_in[best] += indeg[n]
        loads_out[best] += outdeg[n]

    rng = np.random.RandomState(0)
    tgt = N_EDGES // NCORES
    for _ in range(40000):
        mi, mo = loads_in.max(), loads_out.max()
        if mi <= tgt and mo <= tgt:
            break
        if mi - tgt >= mo - tgt:
            hot = int(np.argmax(loads_in))
        else:
            hot = int(np.argmax(loads_out))
        hot_nodes = np.nonzero(assign == hot)[0]
        n1 = int(hot_nodes[rng.randint(NSH)])
        cold = int(np.argmin(loads_in + loads_out))
        if cold == hot:
            break
        cold_nodes = np.nonzero(assign == cold)[0]
        n2 = int(cold_nodes[rng.randint(NSH)])
        di1, do1 = indeg[n1], outdeg[n1]
        di2, do2 = indeg[n2], outdeg[n2]
        new_in = loads_in.copy()
        new_out = loads_out.copy()
        new_in[hot] += di2 - di1
        new_in[cold] += di1 - di2
        new_out[hot] += do2 - do1
        new_out[cold] += do1 - do2
        old_pen = max(loads_in.max() - tgt, 0) + max(loads_out.max() - tgt, 0)
        new_pen = max(new_in.max() - tgt, 0) + max(new_out.max() - tgt, 0)
        if new_pen < old_pen or (new_pen == old_pen and rng.rand() < 0.1):
            assign[n1], assign[n2] = cold, hot
            loads_in, loads_out = new_in, new_out
    nodes_of = [np.nonzero(assign == c)[0] for c in range(NCORES)]
    return nodes_of, int(loads_in.max()), int(loads_out.max())


def _pad128(n):
    return max(((int(n) + P - 1) // P) * P, P)


def _base_j_of(j, ET2):
    b = int(round((j + 0.5) * NT / ET2 - W2 / 2.0))
    return min(max(b, 0), NT - W2)


def _prep_inputs(inputs):
    x = np.asarray(inputs["x"], dtype=np.float32)
    ei = np.asarray(inputs["edge_index"])
    attr = np.asarray(inputs["edge_attr"], dtype=np.float32)
    batch = np.asarray(inputs["batch"]).astype(np.int64)
    src, dst = ei[0].astype(np.int64), ei[1].astype(np.int64)

    nodes_of, max_in, max_out = _balance_nodes(src, dst)
    e_pad1 = _pad128(max_in)
    e_pad2 = _pad128(max_out)
    ET1, ET2 = e_pad1 // P, e_pad2 // P

    core_of = np.empty(N_NODES, dtype=np.int64)
    lidx_of = np.empty(N_NODES, dtype=np.int64)
    for c in range(NCORES):
        core_of[nodes_of[c]] = c
        lidx_of[nodes_of[c]] = np.arange(NSH)
    gid_of = core_of * NSH + lidx_of

    nn1_w = np.asarray(inputs["nn1_w"], dtype=np.float32)
    nn1_b = np.asarray(inputs["nn1_b"], dtype=np.float32)
    nn2_w = np.asarray(inputs["nn2_w"], dtype=np.float32)
    nn2_b = np.asarray(inputs["nn2_b"], dtype=np.float32)
    r1w = np.asarray(inputs["root1_w"], dtype=np.float32)
    b1 = np.asarray(inputs["bias1"], dtype=np.float32)
    r2w = np.asarray(inputs["root2_w"], dtype=np.float32)
    b2 = np.asarray(inputs["bias2"], dtype=np.float32)
    l1w = np.asarray(inputs["lin1_w"], dtype=np.float32)
    l1b = np.asarray(inputs["lin1_b"], dtype=np.float32)
    l2w = np.asarray(inputs["lin2_w"], dtype=np.float32)
    l2b = np.asarray(inputs["lin2_b"], dtype=np.float32)

    w1x = np.zeros((P, 17, H), dtype=np.float32)
    w1r = nn1_w.reshape(16, 2, DN, H)
    w1x[:, :16, :] = w1r.transpose(1, 2, 0, 3).reshape(P, 16, H)
    w1x[:DN, 16, :] = nn1_b.reshape(DN, H)

    w2r = nn2_w.reshape(DE, 2, P, H)
    w2sb = w2r.transpose(2, 0, 1, 3).reshape(P, 64, H)
    b2p = nn2_b.reshape(2, P, H).transpose(1, 0, 2)
    r2x = r2w.reshape(2, P, H).transpose(1, 0, 2)  # [p, kh, o]

    io512f16 = np.tile(np.arange(NSH, dtype=np.float16), (P, 1))
    io_win = np.tile(np.arange(W2 * P, dtype=np.float16), (P, 1))
    iog = np.tile(np.arange(N_GRAPHS, dtype=np.float16), (P, 1))
    iop4 = (np.arange(P)[:, None] + P * np.arange(4)[None, :]).astype(np.float32)
    identb = np.eye(P, dtype=np.float32)

    orig_of_gid = np.empty(N_NODES, dtype=np.int64)
    orig_of_gid[gid_of] = np.arange(N_NODES)
    batch_int = batch[orig_of_gid]
    batchg = batch_int.reshape(NT, P).T.astype(np.float32)

    l1bp = (b2 @ l1w + l1b).reshape(P, 1).astype(np.float32)
    l1wb = l1w.reshape(2, P, P).transpose(1, 0, 2)
    l2wb = l2w.reshape(P, 1)
    l2bp = l2b.reshape(1, 1)

    common = {
        "w1x": w1x.reshape(P, 17 * H).astype(BF),
        "w2sb": w2sb.reshape(P, 64 * H).astype(BF),
        "b2p": b2p.reshape(P, 2 * H).astype(BF),
        "r2x": r2x.reshape(P, 2 * H).astype(BF),
        "io512f16": io512f16,
        "io_win": io_win,
        "iog": iog,
        "iop4": iop4,
        "identb": identb.astype(BF),
        "batchg": batchg,
        "l1bp": l1bp,
        "l1wb": l1wb.reshape(P, 2 * P).astype(BF),
        "l2wb": l2wb.astype(BF),
        "l2b": l2bp,
    }

    in_maps = []
    for c in range(NCORES):
        nodes_c = nodes_of[c]
        # conv1: dst-owned edges sorted by local dst
        e1 = np.nonzero(core_of[dst] == c)[0]
        e1 = e1[np.argsort(lidx_of[dst[e1]], kind="stable")]
        ne1 = len(e1)
        assert ne1 <= e_pad1
        z1 = np.zeros((P, 17, e_pad1), dtype=np.float32)
        xs = x[src[e1]]
        at = attr[e1]
        z = at.reshape(ne1, 16, 2, 1) * xs.reshape(ne1, 1, 1, DN)
        z1[:, :16, :ne1] = z.transpose(2, 3, 1, 0).reshape(P, 16, ne1)
        z1[:DN, 16, :ne1] = xs.T
        d1f = np.full(e_pad1, -1.0, dtype=np.float32)
        d1f[:ne1] = lidx_of[dst[e1]].astype(np.float32)
        dstl1 = d1f.reshape(ET1, P).T.copy()

        xshT = np.zeros((DN + 1, NSH), dtype=np.float32)
        xshT[:DN] = x[nodes_c].T
        xshT[DN] = 1.0
        r1x = np.concatenate([r1w, b1.reshape(1, H)], axis=0)

        # conv2: src-owned edges sorted by internal dst gid, windowed tiles
        e2 = np.nonzero(core_of[src] == c)[0]
        e2 = e2[np.argsort(gid_of[dst[e2]], kind="stable")]
        ne2 = len(e2)
        assert ne2 <= e_pad2
        bounds = [int(round(ne2 * j / ET2)) for j in range(ET2 + 1)]
        bc2 = np.zeros((DE, e_pad2), dtype=np.float32)
        srcl = np.full((1, e_pad2), 30000.0, dtype=np.float32)
        dstl2 = np.full((P, ET2), -999.0, dtype=np.float32)
        for j in range(ET2):
            sel = e2[bounds[j] : bounds[j + 1]]
            nj = len(sel)
            base = _base_j_of(j, ET2)
            gd = gid_of[dst[sel]]
            assert nj == 0 or (gd.min() >= base * P and gd.max() < (base + W2) * P)
            col = j * P
            bc2[:, col : col + nj] = attr[sel].T
            srcl[0, col : col + nj] = lidx_of[src[sel]].astype(np.float32)
            dstl2[:nj, j] = (gd - base * P).astype(np.float32)

        batchl_own = batch[nodes_c].reshape(4, P).T.astype(np.float32)

        m = dict(common)
        m["z1"] = z1.transpose(1, 0, 2).reshape(17 * P, e_pad1).astype(BF)
        m["dstl1"] = dstl1
        m["xshT"] = xshT.astype(BF)
        m["r1x"] = r1x.astype(BF)
        m["bc2"] = bc2.astype(BF)
        m["srcl"] = srcl.astype(np.float16)
        m["dstl2"] = dstl2
        m["batchl_own"] = batchl_own
        in_maps.append(m)
    return (e_pad1, e_pad2), in_maps


# ------------------------------------------------------------------ device
def _build(key, upto="full"):
    e_pad1, e_pad2 = key
    ET1, ET2 = e_pad1 // P, e_pad2 // P
    nc = bacc.Bacc(num_devices=NCORES)

    z1d = nc.dram_tensor("z1", [17 * P, e_pad1], BF16, kind="ExternalInput")
    w1xd = nc.dram_tensor("w1x", [P, 17 * H], BF16, kind="ExternalInput")
    w2d = nc.dram_tensor("w2sb", [P, 64 * H], BF16, kind="ExternalInput")
    b2pd = nc.dram_tensor("b2p", [P, 2 * H], BF16, kind="ExternalInput")
    r2xd = nc.dram_tensor("r2x", [P, 2 * H], BF16, kind="ExternalInput")
    io512d = nc.dram_tensor("io512f16", [P, NSH], F16, kind="ExternalInput")
    iowind = nc.dram_tensor("io_win", [P, W2 * P], F16, kind="ExternalInput")
    iogd = nc.dram_tensor("iog", [P, N_GRAPHS], F16, kind="ExternalInput")
    iop4d = nc.dram_tensor("iop4", [P, 4], F32, kind="ExternalInput")
    identd = nc.dram_tensor("identb", [P, P], BF16, kind="ExternalInput")
    batchgd = nc.dram_tensor("batchg", [P, NT], F32, kind="ExternalInput")
    l1bpd = nc.dram_tensor("l1bp", [P, 1], F32, kind="ExternalInput")
    l1wbd = nc.dram_tensor("l1wb", [P, 2 * P], BF16, kind="ExternalInput")
    l2wbd = nc.dram_tensor("l2wb", [P, 1], BF16, kind="ExternalInput")
    l2bd = nc.dram_tensor("l2b", [1, 1], F32, kind="ExternalInput")
    dstl1d = nc.dram_tensor("dstl1", [P, ET1], F32, kind="ExternalInput")
    xshTd = nc.dram_tensor("xshT", [DN + 1, NSH], BF16, kind="ExternalInput")
    r1xd = nc.dram_tensor("r1x", [DN + 1, H], BF16, kind="ExternalInput")
    bc2d = nc.dram_tensor("bc2", [DE, e_pad2], BF16, kind="ExternalInput")
    srcld = nc.dram_tensor("srcl", [1, e_pad2], F16, kind="ExternalInput")
    dstl2d = nc.dram_tensor("dstl2", [P, ET2], F32, kind="ExternalInput")
    batchownd = nc.dram_tensor("batchl_own", [P, 4], F32, kind="ExternalInput")
    out_shard = nc.dram_tensor("out_shard", [32, 1], F32, kind="ExternalOutput")

    def dbg_out(name, shape):
        return nc.dram_tensor(name, shape, F32, kind="ExternalOutput")

    rg = [list(range(NCORES))]
    touch = {}  # nt -> [(j, s)] compile-time scatter2 schedule
    for j in range(ET2):
        base = _base_j_of(j, ET2)
        for s in range(W2):
            touch.setdefault(base + s, []).append((j, s))

    with tile.TileContext(nc, num_cores=NCORES) as tc:
        with (
            tc.tile_pool(name="const", bufs=1) as cp,
            tc.tile_pool(name="work", bufs=1) as wp,
            tc.tile_pool(name="dram", bufs=1, space="DRAM") as dr,
        ):
            # persistent cross-phase SBUF
            srclsb = cp.tile([P, e_pad2], F16)
            iop4sb = cp.tile([P, 4], F32)
            bc2sb = cp.tile([P, DE, e_pad2], BF16)
            w2sb = cp.tile([P, 64, H], BF16)
            b2sb = cp.tile([P, 2, H], BF16)
            iowin = cp.tile([P, W2 * P], F16)
            dstl2 = cp.tile([P, ET2], F32)
            r2sb = cp.tile([P, 2, H], BF16)
            batchg = cp.tile([P, NT], F32)
            batchown = cp.tile([P, 4], F32)
            iogsb = cp.tile([P, N_GRAPHS], F16)
            identsb = cp.tile([P, P], BF16)
            l1wsb = cp.tile([P, 2, P], BF16)
            l1bpsb = cp.tile([P, 1], F32)
            l2wsb = cp.tile([P, 1], BF16)
            l2bsb = cp.tile([1, 1], F32)
            onesb = cp.tile([P, 1], BF16)
            h1sb = cp.tile([P, 4, H], BF16)
            h1srcT = cp.tile([P, 2, e_pad2], BF16)
            h1shT = cp.tile([P, 8, P], BF16)  # slot = lt*2+kh

            with tc.tile_pool(name="c1p", bufs=1) as c1p:
                z1t = [
                    c1p.tile([P, e_pad1], BF16, name=f"z1t{t}") for t in range(17)
                ]
                w1sb = c1p.tile([P, 17, H], BF16)
                io512 = c1p.tile([P, NSH], F16)
                dstl1 = c1p.tile([P, ET1], F32)
                xshsb = c1p.tile([DN + 1, NSH], BF16)
                r1xsb = c1p.tile([DN + 1, H], BF16)

                # ---- DMA program order (sync engine, JIT priority)
                z1v = z1d[:].rearrange("(t p) e -> t p e", p=P)
                for t in range(2):
                    nc.sync.dma_start(out=z1t[t][:], in_=z1v[t])
                w1v = w1xd[:].rearrange("p (t o) -> p t o", o=H)
                for cch in range(4):
                    nc.sync.dma_start(
                        out=w1sb[:, 4 * cch : 4 * (cch + 1), :],
                        in_=w1v[:, 4 * cch : 4 * (cch + 1), :],
                    )
                    nc.sync.dma_start(out=z1t[2 + cch][:], in_=z1v[2 + cch])
                nc.sync.dma_start(out=w1sb[:, 16:17, :], in_=w1v[:, 16:17, :])
                nc.sync.dma_start(out=io512[:], in_=io512d[:])
                nc.sync.dma_start(out=dstl1[:], in_=dstl1d[:])
                nc.sync.dma_start(out=xshsb[:], in_=xshTd[:])
                nc.sync.dma_start(out=r1xsb[:], in_=r1xd[:])
                nc.sync.dma_start(
                    out=srclsb[:], in_=srcld[0:1, :].partition_broadcast(P)
                )
                nc.sync.dma_start(out=iop4sb[:], in_=iop4d[:])
                for t in range(6, 17):
                    nc.sync.dma_start(out=z1t[t][:], in_=z1v[t])
                for k in range(DE):
                    nc.sync.dma_start(
                        out=bc2sb[:, k : k + 1, :],
                        in_=bc2d[k : k + 1, :].partition_broadcast(P),
                    )
                w2v = w2d[:].rearrange("p (t o) -> p t o", o=H)
                for cch in range(8):
                    nc.sync.dma_start(
                        out=w2sb[:, 8 * cch : 8 * (cch + 1), :],
                        in_=w2v[:, 8 * cch : 8 * (cch + 1), :],
                    )
                nc.sync.dma_start(
                    out=b2sb[:], in_=b2pd[:].rearrange("p (t o) -> p t o", o=H)
                )
                nc.sync.dma_start(out=iowin[:], in_=iowind[:])
                nc.sync.dma_start(out=dstl2[:], in_=dstl2d[:])
                nc.sync.dma_start(
                    out=r2sb[:], in_=r2xd[:].rearrange("p (t o) -> p t o", o=H)
                )
                nc.sync.dma_start(out=batchg[:], in_=batchgd[:])
                nc.sync.dma_start(out=batchown[:], in_=batchownd[:])
                nc.sync.dma_start(out=iogsb[:], in_=iogd[:])
                nc.sync.dma_start(out=identsb[:], in_=identd[:])
                nc.sync.dma_start(
                    out=l1wsb[:], in_=l1wbd[:].rearrange("p (t o) -> p t o", o=P)
                )
                nc.sync.dma_start(out=l1bpsb[:], in_=l1bpd[:])
                nc.sync.dma_start(out=l2wsb[:], in_=l2wbd[:])
                nc.sync.dma_start(out=l2bsb[:], in_=l2bd[:])
                nc.vector.memset(onesb[:], 1.0)

                # ============ conv1 ============
                with tc.tile_pool(name="ps1", bufs=1, space="PSUM") as ps1:
                    msg_ps = [
                        ps1.tile([P, 2 * H], F32, space="PSUM", name=f"ms{i}")
                        for i in range((ET1 + 1) // 2)
                    ]

                    def m1(j):
                        return msg_ps[j // 2][:, (j % 2) * H : (j % 2) * H + H]

                    agg_ps = [
                        ps1.tile([P, 2 * H], F32, space="PSUM", name=f"ag{i}")
                        for i in range(2)
                    ]

                    def a1(n):
                        return agg_ps[n // 2][:, (n % 2) * H : (n % 2) * H + H]

                    for t in range(17):
                        for j in range(ET1):
                            nc.tensor.matmul(
                                m1(j),
                                lhsT=z1t[t][:, P * j : P * (j + 1)],
                                rhs=w1sb[:, t, :],
                                start=(t == 0 and j % 2 == 0),
                                stop=(t == 16),
                                skip_group_check=True,
                            )

                    oh1 = [
                        c1p.tile([P, P], BF16, name=f"oh1_{j}_{n}")
                        for j in range(ET1)
                        for n in range(4)
                    ]
                    for j in range(ET1):
                        for n in range(4):
                            nc.vector.tensor_scalar(
                                out=oh1[j * 4 + n][:],
                                in0=io512[:, P * n : P * (n + 1)],
                                scalar1=dstl1[:, j : j + 1],
                                scalar2=None,
                                op0=ALU.is_equal,
                            )

                    msbs1 = [
                        c1p.tile([P, H], BF16, name=f"msb1_{j}") for j in range(ET1)
                    ]
                    for j in range(ET1):
                        nc.scalar.activation(
                            out=msbs1[j][:], in_=m1(j), func=AF.Copy
                        )
                    if upto == "msg1":
                        dm1 = dbg_out("d_msg1", [P, ET1 * H])
                        for j in range(ET1):
                            tmpj = wp.tile([P, H], F32, tag="dbgm1")
                            nc.vector.tensor_copy(out=tmpj[:], in_=msbs1[j][:])
                            nc.sync.dma_start(
                                out=dm1[:, H * j : H * (j + 1)], in_=tmpj[:]
                            )
                        do1 = dbg_out("d_oh1", [P, 4 * P])
                        for n in range(4):
                            tmpo = wp.tile([P, P], F32, tag="dbgo1")
                            nc.vector.tensor_copy(out=tmpo[:], in_=oh1[n][:])
                            nc.sync.dma_start(
                                out=do1[:, P * n : P * (n + 1)], in_=tmpo[:]
                            )
                    for j in range(ET1):
                        for n in range(4):
                            nc.tensor.matmul(
                                a1(n),
                                lhsT=oh1[j * 4 + n][:],
                                rhs=msbs1[j][:],
                                start=(j == 0 and n % 2 == 0),
                                stop=False,
                                skip_group_check=True,
                            )
                    for n in range(4):
                        nc.tensor.matmul(
                            a1(n),
                            lhsT=xshsb[:, P * n : P * (n + 1)],
                            rhs=r1xsb[:],
                            start=False,
                            stop=True,
                            skip_group_check=True,
                        )
                    for q in range(2):
                        nc.scalar.activation(
                            out=h1sb[:, 2 * q : 2 * q + 2, :],
                            in_=agg_ps[q][:, 0 : 2 * H],
                            func=AF.Relu,
                        )

            if upto == "h1":
                dh = dbg_out("d_h1", [P, 4 * H])
                tmp = wp.tile([P, 4, H], F32, tag="dbgf")
                nc.vector.tensor_copy(out=tmp[:], in_=h1sb[:])
                nc.sync.dma_start(
                    out=dh[:].rearrange("p (t o) -> p t o", o=H), in_=tmp[:]
                )

            # ============ transition: h1srcT gather + h1shT transposes
            with tc.tile_pool(name="tpp", bufs=1) as tpool:
                src_oh = [
                    tpool.tile([P, e_pad2], BF16, name=f"soh{lt}")
                    for lt in range(4)
                ]
                for lt in range(4):
                    nc.vector.tensor_scalar(
                        out=src_oh[lt][:],
                        in0=srclsb[:],
                        scalar1=iop4sb[:, lt : lt + 1],
                        scalar2=None,
                        op0=ALU.is_equal,
                    )
                # gather column chunks of up to 512
                chunks = []
                c0 = 0
                while c0 < e_pad2:
                    w = min(2 * H, e_pad2 - c0)
                    chunks.append((c0, w))
                    c0 += w
                with tc.tile_pool(name="ps2", bufs=1, space="PSUM") as ps2:
                    gat = [
                        ps2.tile([P, 2 * H], F32, space="PSUM", name=f"gt{i}")
                        for i in range(2 * len(chunks))
                    ]
                    for ih in range(2):
                        for ci, (c0, w) in enumerate(chunks):
                            g = gat[ih * len(chunks) + ci]
                            for lt in range(4):
                                nc.tensor.matmul(
                                    g[:, 0:w],
                                    lhsT=h1sb[:, lt, P * ih : P * (ih + 1)],
                                    rhs=src_oh[lt][:, c0 : c0 + w],
                                    start=(lt == 0),
                                    stop=(lt == 3),
                                    skip_group_check=True,
                                )
                    for ih in range(2):
                        for ci, (c0, w) in enumerate(chunks):
                            nc.scalar.activation(
                                out=h1srcT[:, ih, c0 : c0 + w],
                                in_=gat[ih * len(chunks) + ci][:, 0:w],
                                func=AF.Copy,
                            )
                    tpp_ps = [
                        ps2.tile([P, P], BF16, space="PSUM", name=f"tp{i}")
                        for i in range(2)
                    ]
                    for lt in range(4):
                        for kh in range(2):
                            t = tpp_ps[(lt * 2 + kh) % 2]
                            nc.tensor.transpose(
                                out=t[:],
                                in_=h1sb[:, lt, P * kh : P * (kh + 1)],
                                identity=identsb[:],
                            )
                            nc.scalar.activation(
                                out=h1shT[:, lt * 2 + kh, :], in_=t[:], func=AF.Copy
                            )

            # ============ conv2 ============
            with tc.tile_pool(name="c2p", bufs=1) as c2p:
                h2sb = c2p.tile([P, NT, H], BF16)
                rootloc = c2p.tile([P, 4, H], BF16)
                plsb = c2p.tile([P, 2, H + 1], F32)
                msbs2 = [
                    c2p.tile([P, H], BF16, name=f"msb2_{j}") for j in range(ET2)
                ]
                with tc.tile_pool(name="ps3", bufs=1, space="PSUM") as ps3:
                    msg2_ps = [
                        ps3.tile([P, 2 * H], F32, space="PSUM", name=f"m2_{i}")
                        for i in range((ET2 + 1) // 2)
                    ]

                    def m2(j):
                        return msg2_ps[j // 2][:, (j % 2) * H : (j % 2) * H + H]

                    for t in range(64):
                        k, ih = t // 2, t % 2
                        zt = wp.tile([P, e_pad2], BF16, tag="zt", bufs=3)
                        nc.vector.tensor_tensor(
                            out=zt[:],
                            in0=h1srcT[:, ih, :],
                            in1=bc2sb[:, k, :],
                            op=ALU.mult,
                        )
                        for j in range(ET2):
                            nc.tensor.matmul(
                                m2(j),
                                lhsT=zt[:, P * j : P * (j + 1)],
                                rhs=w2sb[:, t, :],
                                start=(t == 0 and j % 2 == 0),
                                stop=False,
                                skip_group_check=True,
                            )
                    for j in range(ET2):
                        for ih in range(2):
                            nc.tensor.matmul(
                                m2(j),
                                lhsT=h1srcT[:, ih, P * j : P * (j + 1)],
                                rhs=b2sb[:, ih, :],
                                start=False,
                                stop=(ih == 1),
                                skip_group_check=True,
                            )

                    if upto == "msg2":
                        dm = dbg_out("d_msg2", [P, ET2 * H])
                        for i in range((ET2 + 1) // 2):
                            w = min(2 * H, (ET2 - 2 * i) * H)
                            tmpm = wp.tile([P, 2 * H], F32, tag="dbgm")
                            nc.scalar.activation(
                                out=tmpm[:, 0:w],
                                in_=msg2_ps[i][:, 0:w],
                                func=AF.Copy,
                            )
                            nc.sync.dma_start(
                                out=dm[:, 2 * H * i : 2 * H * i + w],
                                in_=tmpm[:, 0:w],
                            )

                    h2_ps = [
                        ps3.tile([P, 2 * H], F32, space="PSUM", name=f"h2_{i}")
                        for i in range(3)
                    ]

                    def h2p(nt):
                        q = nt % 6
                        return h2_ps[q // 2][:, (q % 2) * H : (q % 2) * H + H]

                    # start only on the first matmul touching each bank per
                    # reuse round (start marks the whole 2KB zero-region)
                    h2_start = {}
                    for b in range(3):
                        for k0 in range(0, NT, 6):
                            pair = [
                                n
                                for n in (k0 + 2 * b, k0 + 2 * b + 1)
                                if n in touch
                            ]
                            if not pair:
                                continue
                            fnt = min(pair, key=lambda n: touch[n][0])
                            h2_start[(fnt, touch[fnt][0])] = True

                    for j in range(ET2):
                        nc.scalar.activation(
                            out=msbs2[j][:], in_=m2(j), func=AF.Copy
                        )
                        base = _base_j_of(j, ET2)
                        for s in range(W2):
                            nt = base + s
                            last = touch[nt][-1] == (j, s)
                            oh2 = wp.tile([P, P], BF16, tag="oh2", bufs=8)
                            nc.vector.tensor_scalar(
                                out=oh2[:],
                                in0=iowin[:, P * s : P * (s + 1)],
                                scalar1=dstl2[:, j : j + 1],
                                scalar2=None,
                                op0=ALU.is_equal,
                            )
                            nc.tensor.matmul(
                                h2p(nt),
                                lhsT=oh2[:],
                                rhs=msbs2[j][:],
                                start=h2_start.get((nt, (j, s)), False),
                                stop=last,
                                skip_group_check=True,
                            )
                            if last:
                                nc.scalar.activation(
                                    out=h2sb[:, nt, :], in_=h2p(nt), func=AF.Copy
                                )

                if upto == "h2":
                    dh2 = dbg_out("d_h2", [P, NT * H])
                    for q in range(4):
                        tmp = wp.tile([P, 8, H], F32, tag="dbgf")
                        nc.vector.tensor_copy(
                            out=tmp[:], in_=h2sb[:, 8 * q : 8 * (q + 1), :]
                        )
                        nc.sync.dma_start(
                            out=dh2[:].rearrange("p (t o) -> p t o", o=H)[
                                :, 8 * q : 8 * (q + 1), :
                            ],
                            in_=tmp[:],
                        )

                # ===== own root + partial pooling + RS + readout
                with tc.tile_pool(name="ps4", bufs=1, space="PSUM") as ps4:
                    root_ps = [
                        ps4.tile([P, 2 * H], F32, space="PSUM", name=f"rt{i}")
                        for i in range(2)
                    ]
                    for lt in range(4):
                        rp = root_ps[lt // 2][:, (lt % 2) * H : (lt % 2) * H + H]
                        for kh in range(2):
                            nc.tensor.matmul(
                                rp,
                                lhsT=h1shT[:, lt * 2 + kh, :],
                                rhs=r2sb[:, kh, :],
                                start=(kh == 0 and lt % 2 == 0),
                                stop=(kh == 1),
                                skip_group_check=True,
                            )
                    for lt in range(4):
                        nc.scalar.activation(
                            out=rootloc[:, lt, :],
                            in_=root_ps[lt // 2][
                                :, (lt % 2) * H : (lt % 2) * H + H
                            ],
                            func=AF.Copy,
                        )

                    pool_ps = [
                        ps4.tile([P, 2 * H], F32, space="PSUM", name=f"pl{g}")
                        for g in range(2)
                    ]
                    for nt in range(NT):
                        for gt in range(2):
                            ohp = wp.tile([P, P], BF16, tag="ohp", bufs=8)
                            nc.vector.tensor_scalar(
                                out=ohp[:],
                                in0=iogsb[:, P * gt : P * (gt + 1)],
                                scalar1=batchg[:, nt : nt + 1],
                                scalar2=None,
                                op0=ALU.is_equal,
                            )
                            nc.tensor.matmul(
                                pool_ps[gt][:, 0:H],
                                lhsT=ohp[:],
                                rhs=h2sb[:, nt, :],
                                start=(nt == 0),
                                stop=False,
                                skip_group_check=True,
                            )
                    for lt in range(4):
                        for gt in range(2):
                            oho = wp.tile([P, P], BF16, tag="ohp", bufs=8)
                            nc.vector.tensor_scalar(
                                out=oho[:],
                                in0=iogsb[:, P * gt : P * (gt + 1)],
                                scalar1=batchown[:, lt : lt + 1],
                                scalar2=None,
                                op0=ALU.is_equal,
                            )
                            nc.tensor.matmul(
                                pool_ps[gt][:, 0:H],
                                lhsT=oho[:],
                                rhs=rootloc[:, lt, :],
                                start=False,
                                stop=(lt == 3),
                                skip_group_check=True,
                            )
                            nc.tensor.matmul(
                                pool_ps[gt][:, H : H + 1],
                                lhsT=oho[:],
                                rhs=onesb[:],
                                start=False,
                                stop=(lt == 3),
                                skip_group_check=True,
                            )
                    for gt in range(2):
                        nc.scalar.activation(
                            out=plsb[:, gt, :],
                            in_=pool_ps[gt][:, 0 : H + 1],
                            func=AF.Copy,
                        )

                    if upto == "pool":
                        dp = dbg_out("d_pool", [P, 2 * (H + 1)])
                        nc.sync.dma_start(
                            out=dp[:].rearrange("p (t o) -> p t o", o=H + 1),
                            in_=plsb[:],
                        )

                    pcc_in = dr.tile([N_GRAPHS, H + 1], F32)
                    nc.sync.dma_start(
                        out=pcc_in[:].rearrange("(g p) c -> p g c", p=P),
                        in_=plsb[:],
                    )
                    pcc_out = dr.tile([32, H + 1], F32)
                    nc.gpsimd.collective_compute(
                        "ReduceScatter",
                        ALU.add,
                        replica_groups=rg,
                        ins=[pcc_in[:].opt()],
                        outs=[pcc_out[:].opt()],
                    )

                    # readout on the 32 local graphs
                    pl = c2p.tile([32, H + 1], F32)
                    nc.sync.dma_start(out=pl[:], in_=pcc_out[:])
                    cntt = wp.tile([32, 1], F32, tag="cnt")
                    nc.vector.tensor_scalar_max(cntt[:], pl[:, H : H + 1], 1.0)
                    rec = wp.tile([32, 1], F32, tag="rec")
                    nc.vector.reciprocal(out=rec[:], in_=cntt[:])
                    meanbf = wp.tile([32, H], BF16, tag="meanbf")
                    nc.vector.tensor_scalar(
                        out=meanbf[:],
                        in0=pl[:, 0:H],
                        scalar1=rec[:, :1],
                        scalar2=None,
                        op0=ALU.mult,
                    )
                    meanT = wp.tile([P, 2, 32], BF16, tag="meanT")
                    for kh in range(2):
                        tpm = ps4.tile([P, 32], BF16, space="PSUM", tag="tpm")
                        nc.tensor.transpose(
                            out=tpm[:],
                            in_=meanbf[:, P * kh : P * (kh + 1)],
                            identity=identsb[0:32, 0:32],
                        )
                        nc.scalar.activation(
                            out=meanT[:, kh, :], in_=tpm[:], func=AF.Copy
                        )
                    z1_ps = ps4.tile([P, 32], F32, space="PSUM", tag="z1ps")
                    for kh in range(2):
                        nc.tensor.matmul(
                            z1_ps[:],
                            lhsT=l1wsb[:, kh, :],
                            rhs=meanT[:, kh, :],
                            start=(kh == 0),
                            stop=(kh == 1),
                        )
                    z1out = wp.tile([P, 32], BF16, tag="z1sb")
                    nc.scalar.activation(
                        out=z1out[:], in_=z1_ps[:], func=AF.Relu, bias=l1bpsb[:, :1]
                    )
                    o_ps = ps4.tile([1, 32], F32, space="PSUM", tag="ops")
                    nc.tensor.matmul(
                        o_ps[:], lhsT=l2wsb[:], rhs=z1out[:], start=True, stop=True
                    )
                    osb = wp.tile([1, 32], F32, tag="osb")
                    nc.scalar.activation(
                        out=osb[:], in_=o_ps[:], func=AF.Sigmoid, bias=l2bsb[:, :1]
                    )
                    nc.sync.dma_start(
                        out=out_shard[:].rearrange("g one -> one g"), in_=osb[:]
                    )

    nc.compile()
    return nc


def kernel(**inputs) -> np.ndarray:
    key, in_maps = _prep_inputs(inputs)
    if key not in _cache:
        _cache[key] = _build(key)
    nc = _cache[key]
    res = bass_utils.run_bass_kernel_spmd(nc, in_maps, core_ids=list(range(NCORES)))
    return np.concatenate(
        [
            np.asarray(res.results[c]["out_shard"], dtype=np.float32)
            for c in range(NCORES)
        ],
        axis=0,
    )


def run_debug(upto, **inputs):
    key, in_maps = _prep_inputs(inputs)
    nc = _build(key, upto=upto)
    res = bass_utils.run_bass_kernel_spmd(nc, in_maps, core_ids=list(range(NCORES)))
    return key, res
